# revision 24
# baseline (speedup 1.0000x reference)
"""Trainium2 Bass kernel for nn_PromptCompressorLightweight (iteration 2).

Head-parallel SPMD over 8 NeuronCores (1 KV head per core), row-major
t-layout (t = p*16 + r):
  - attn weighted column-sum on PE (w stationary, attn streaming)
  - k/v/q norms from single 1MB contiguous loads, square+reduce on DVE
  - x-norm sequence-sharded across cores + AllGather
  - tiny per-head MLP on DVE
  - exact top-512: f32 bisection + tie fill; positions via strict-upper
    matmul (partition scan) + shifted-add free scan; keep indices
    recovered by compare-sum against broadcast position array (no
    scatter); k/v rows fetched with 8 indirect gathers.
"""
import os

import numpy as np

import concourse.bass as bass
import concourse.mybir as mybir
import concourse.tile as tile
from concourse.bass_utils import run_bass_kernel_spmd
from concourse.masks import make_identity, make_upper_triangular

B, H, HQ, T, D, M = 1, 8, 32, 2048, 128, 4096
G = HQ // H
NF, HID = 5, 10
L = 512
P, F = 128, 16       # row-major: t = p*16 + r
W = 8
TS = T // W
N_MULTI = 5          # 128-way threshold-search rounds
N_POLISH = 3         # binary polish rounds (to 1 ulp)
BIG = 8192.0

f32 = mybir.dt.float32
i32 = mybir.dt.int32

last_results = None


def _fix_multi_waits(nc):
    """This container's walrus rejects >1 sync wait per instruction. Move
    extra waits onto nop carriers inserted just before, on the same engine."""
    n_split = 0
    for f in nc.m.functions:
        for bb in f.blocks:
            insts = bb.instructions
            i = 0
            while i < len(insts):
                inst = insts[i]
                si = getattr(inst, "sync_info", None)
                waits = list(si.on_wait) if si is not None and si.on_wait else []
                if len(waits) > 1:
                    inst.sync_info = mybir.SyncInfo(
                        on_wait=waits[-1:],
                        on_update=list(si.on_update) if si.on_update else [],
                    )
                    for k, wt in enumerate(waits[:-1]):
                        nop = mybir.InstNoOp(
                            name=f"waitfix-{n_split}-{k}", ins=[], outs=[])
                        nop.engine = inst.engine
                        nop.sync_info = mybir.SyncInfo(on_wait=[wt], on_update=[])
                        nc.register_instruction(nop, overwrite=True)
                        insts.insert(i, nop)
                        i += 1
                    n_split += 1
                i += 1
    return n_split


def _bcast_mid(ap, n_mid):
    """[P, k] AP -> [P, n_mid, k] with a step-0 middle dim."""
    return bass.AP(ap.tensor, ap.offset, [ap.ap[0], [0, n_mid], ap.ap[1]])


def _build():
    nc = bass.Bass("TRN2", num_devices=W)

    attn_in = nc.dram_tensor("attn", [T, T], f32, kind="ExternalInput")
    k_in = nc.dram_tensor("k", [T, D], f32, kind="ExternalInput")
    v_in = nc.dram_tensor("v", [T, D], f32, kind="ExternalInput")
    q_in = nc.dram_tensor("q", [G, T, D], f32, kind="ExternalInput")
    x_in = nc.dram_tensor("xs", [TS, M], f32, kind="ExternalInput")
    w1r_in = nc.dram_tensor("w1r", [P, HID * NF], f32, kind="ExternalInput")
    b1r_in = nc.dram_tensor("b1r", [P, HID], f32, kind="ExternalInput")
    w2r_in = nc.dram_tensor("w2r", [P, HID], f32, kind="ExternalInput")
    b2r_in = nc.dram_tensor("b2r", [P, 1], f32, kind="ExternalInput")

    debug = bool(int(os.environ.get("KDEBUG", "0")))
    keep_out = nc.dram_tensor("keep", [L, 1], i32, kind="ExternalOutput")
    ko_out = nc.dram_tensor("ko", [L, D], f32, kind="ExternalOutput")
    vo_out = nc.dram_tensor("vo", [L, D], f32, kind="ExternalOutput")
    if debug:
        dbg_feats = nc.dram_tensor("dbg_feats", [P, F * NF], f32, kind="ExternalOutput")
        dbg_pri = nc.dram_tensor("dbg_pri", [P, F], f32, kind="ExternalOutput")
        dbg_bounds = nc.dram_tensor("dbg_bounds", [P, 2], f32, kind="ExternalOutput")
        dbg_sel = nc.dram_tensor("dbg_sel", [P, F], f32, kind="ExternalOutput")
        dbg_offf = nc.dram_tensor("dbg_offf", [P, F], f32, kind="ExternalOutput")
        dbg_keepf = nc.dram_tensor("dbg_keepf", [P, 4], f32, kind="ExternalOutput")

    as_dram = nc.dram_tensor("as_dram", [1, T], f32)
    pri_dram = nc.dram_tensor("pri_dram", [1, T], f32)
    pos_dram = nc.dram_tensor("pos_dram", [1, T], f32)
    cc_in = nc.dram_tensor("cc_in", [1, TS], f32)
    cc_out = nc.dram_tensor("cc_out", [1, T], f32, addr_space="Shared")

    # inline consts
    wrow_np = (np.arange(F)[None, :] * P + np.arange(P)[:, None] + 1).astype(np.float32)
    wrow_d = nc.inline_tensor(wrow_np, "wrow")   # attn row weights, i = c*128+p
    recipd_d = nc.inline_tensor(
        (1.0 / np.arange(T, 0, -1, dtype=np.float32))[None, :], "recipd")
    tio_d = nc.inline_tensor(np.arange(T, dtype=np.float32)[None, :], "tiorow")
    qiota_np = (np.arange(P)[:, None] * 4 + np.arange(4)[None, :]).astype(np.float32)
    qiota_d = nc.inline_tensor(qiota_np, "qiota")  # q = p*4 + j
    mstep_d = nc.inline_tensor(
        ((np.arange(P) + 1.0) / 129.0).astype(np.float32)[:, None], "mstep")

    with tile.TileContext(nc) as tc:
        with tc.tile_pool(name="const", bufs=1) as cpool, \
             tc.tile_pool(name="attn", bufs=4) as apool, \
             tc.tile_pool(name="big", bufs=1) as bpool, \
             tc.tile_pool(name="xp", bufs=1) as xpool, \
             tc.tile_pool(name="io", bufs=4) as iopool, \
             tc.tile_pool(name="sm", bufs=2) as sm, \
             tc.tile_pool(name="st", bufs=1) as st:

            # ---------------- constants ----------------
            SU128 = cpool.tile([P, P], f32, tag="su128")
            make_upper_triangular(nc, SU128[:], val=1.0, diag=False)
            ID128 = cpool.tile([P, P], f32, tag="id128")
            make_identity(nc, ID128[:])
            I2 = cpool.tile([2, 2], f32, tag="i2")
            make_identity(nc, I2[:])
            ones128 = cpool.tile([P, P], f32, tag="ones128")
            nc.vector.memset(ones128[:], 1.0)
            onesrow = cpool.tile([1, P], f32, tag="onesrow")
            nc.vector.memset(onesrow[:], 1.0)
            wrow = cpool.tile([P, F], f32, tag="wrow")
            nc.scalar.dma_start(out=wrow[:], in_=wrow_d[:])
            recipd = cpool.tile([1, T], f32, tag="recipd")
            nc.scalar.dma_start(out=recipd[:], in_=recipd_d[:])
            qiota = cpool.tile([P, 4], f32, tag="qiota")
            nc.scalar.dma_start(out=qiota[:], in_=qiota_d[:])
            mstep = cpool.tile([P, 1], f32, tag="mstep")
            nc.scalar.dma_start(out=mstep[:], in_=mstep_d[:])
            tio_rep = cpool.tile([P, T], f32, tag="tiorep")
            nc.scalar.dma_start(out=tio_rep[:],
                              in_=bass.AP(tio_d, 0, [[0, P], [1, T]]))
            w1r = cpool.tile([P, HID * NF], f32, tag="w1r")
            nc.scalar.dma_start(out=w1r[:], in_=w1r_in[:])
            b1r = cpool.tile([P, HID], f32, tag="b1r")
            nc.scalar.dma_start(out=b1r[:], in_=b1r_in[:])
            w2r = cpool.tile([P, HID], f32, tag="w2r")
            nc.scalar.dma_start(out=w2r[:], in_=w2r_in[:])
            b2r = cpool.tile([P, 1], f32, tag="b2r")
            nc.scalar.dma_start(out=b2r[:], in_=b2r_in[:])

            feats = st.tile([P, F * NF], f32, tag="feats")  # [128,16,5]
            feats3 = feats[:].rearrange("p (c f) -> p c f", f=NF)
            trash = st.tile([P, M], f32, tag="bigtrash")  # shared scratch
            trash2 = trash[:, :F * D]
            trash3 = trash2.rearrange("p (r d) -> p r d", d=D)

            # ---------------- x norm (sharded) + allgather ----------------
            hp_ctx = tc.high_priority()
            hp_ctx.__enter__()
            xn = st.tile([P, 2], f32, tag="xn")
            for c in range(2):
                xc = xpool.tile([P, M], f32, tag="xc", bufs=2)
                nc.sync.dma_start(out=xc[:], in_=x_in[c * P:(c + 1) * P, :])
                nc.vector.scalar_tensor_tensor(
                    out=trash[:], in0=xc[:], scalar=0.0, in1=xc[:],
                    op0=mybir.AluOpType.add, op1=mybir.AluOpType.mult,
                    accum_out=xn[:, c:c + 1])
            xnr = st.tile([P, 2], f32, tag="xnr")
            nc.scalar.sqrt(xnr[:], xn[:])
            for c in range(2):
                nc.gpsimd.dma_start(out=cc_in[0, c * P:(c + 1) * P],
                                    in_=xnr[:, c:c + 1])
            nc.gpsimd.collective_compute(
                "AllGather", mybir.AluOpType.bypass,
                replica_groups=[list(range(W))],
                ins=[cc_in[:]], outs=[cc_out[:]])
            cc_rm = cc_out[:].rearrange("one (p r) -> one p r", p=P, r=F)
            nc.scalar.dma_start(out=feats3[:, :, 4], in_=cc_rm[0])
            hp_ctx.__exit__(None, None, None)

            # ---------------- attn weighted colsum ----------------
            with tc.tile_pool(name="psA", bufs=1, space="PSUM") as psA:
                as_ps = [psA.tile([1, 512], f32, tag=f"asps{n}",
                                  name=f"asps{n}") for n in range(4)]
                for c in range(F):
                    at = apool.tile([P, T], f32, tag="at")
                    nc.sync.dma_start(out=at[:],
                                      in_=attn_in[c * P:(c + 1) * P, :])
                    for n in range(4):
                        nc.tensor.matmul(
                            out=as_ps[n][:], lhsT=wrow[:, c:c + 1],
                            rhs=at[:, n * 512:(n + 1) * 512],
                            start=(c == 0), stop=(c == F - 1))
                as_row = st.tile([1, T], f32, tag="asrow")
                for n in range(4):
                    nc.vector.tensor_tensor(
                        out=as_row[:, n * 512:(n + 1) * 512], in0=as_ps[n][:],
                        in1=recipd[:, n * 512:(n + 1) * 512],
                        op=mybir.AluOpType.mult)
            nc.scalar.dma_start(out=as_dram[:], in_=as_row[:])
            as_rm = as_dram[:].rearrange("one (p r) -> one p r", p=P, r=F)
            nc.scalar.dma_start(out=feats3[:, :, 0], in_=as_rm[0])

            # ---------------- k/v/q norms (1MB contiguous loads) ----------
            for name_, src, fi in (("kn", k_in, 1), ("vn", v_in, 2)):
                kb = bpool.tile([P, F * D], f32, tag="kb", bufs=2)
                kb3 = kb[:].rearrange("p (r d) -> p r d", d=D)
                nc.sync.dma_start(
                    out=kb3, in_=src[:].rearrange("(p r) d -> p r d", p=P, r=F))
                nc.vector.tensor_tensor(out=trash2, in0=kb[:], in1=kb[:],
                                        op=mybir.AluOpType.mult)
                n2 = st.tile([P, F], f32, tag=f"{name_}2", name=f"{name_}2")
                nc.vector.tensor_reduce(out=n2[:], in_=trash3,
                                        axis=mybir.AxisListType.X,
                                        op=mybir.AluOpType.add)
                nc.scalar.sqrt(feats3[:, :, fi], n2[:])
            qs = []
            for g in range(G):
                qt = bpool.tile([P, F * D], f32, tag=f"qt{g}", name=f"qt{g}")
                nc.sync.dma_start(
                    out=qt[:].rearrange("p (r d) -> p r d", d=D),
                    in_=q_in[g].rearrange("(p r) d -> p r d", p=P, r=F))
                qs.append(qt)
            nc.vector.tensor_tensor(out=qs[0][:], in0=qs[0][:], in1=qs[1][:],
                                    op=mybir.AluOpType.add)
            nc.vector.tensor_tensor(out=qs[2][:], in0=qs[2][:], in1=qs[3][:],
                                    op=mybir.AluOpType.add)
            nc.vector.tensor_tensor(out=qs[0][:], in0=qs[0][:], in1=qs[2][:],
                                    op=mybir.AluOpType.add)
            nc.vector.tensor_tensor(out=trash2, in0=qs[0][:], in1=qs[0][:],
                                    op=mybir.AluOpType.mult)
            qn2 = st.tile([P, F], f32, tag="qn2")
            nc.vector.tensor_reduce(out=qn2[:], in_=trash3,
                                    axis=mybir.AxisListType.X,
                                    op=mybir.AluOpType.add)
            nc.scalar.activation(feats3[:, :, 3], qn2[:],
                                 mybir.ActivationFunctionType.Sqrt,
                                 scale=1.0 / (G * G))

            # ---------------- MLP ----------------
            h3d = st.tile([P, F * HID], f32, tag="h3d")
            h3 = h3d[:].rearrange("p (c j) -> p c j", j=HID)
            mul5 = sm.tile([P, F * NF], f32, tag="mul5")
            mul5_3 = mul5[:].rearrange("p (c f) -> p c f", f=NF)
            for j in range(HID):
                w1j = _bcast_mid(w1r[:, j * NF:(j + 1) * NF], F)
                nc.vector.tensor_tensor(out=mul5_3, in0=feats3, in1=w1j,
                                        op=mybir.AluOpType.mult)
                nc.vector.tensor_reduce(
                    out=h3[:, :, j], in_=mul5_3, axis=mybir.AxisListType.X,
                    op=mybir.AluOpType.add)
            b1b = _bcast_mid(b1r[:], F)
            nc.vector.tensor_tensor(out=h3, in0=h3, in1=b1b,
                                    op=mybir.AluOpType.add)
            nc.scalar.activation(h3d[:], h3d[:],
                                 mybir.ActivationFunctionType.Relu)
            mul10 = sm.tile([P, F * HID], f32, tag="mul10")
            mul10_3 = mul10[:].rearrange("p (c j) -> p c j", j=HID)
            w2b = _bcast_mid(w2r[:], F)
            nc.vector.tensor_tensor(out=mul10_3, in0=h3, in1=w2b,
                                    op=mybir.AluOpType.mult)
            pri = st.tile([P, F], f32, tag="pri")
            nc.vector.tensor_reduce(out=pri[:], in_=mul10_3,
                                    axis=mybir.AxisListType.X,
                                    op=mybir.AluOpType.add)
            nc.vector.tensor_scalar(out=pri[:], in0=pri[:],
                                    scalar1=b2r[:, 0:1], scalar2=None,
                                    op0=mybir.AluOpType.add)
            # replicate priorities to [128, T] (each partition = full array)
            nc.scalar.dma_start(out=pri_dram[0:1, :].rearrange(
                "one (p r) -> one p r", p=P, r=F)[0], in_=pri[:])
            pri_rep = st.tile([P, T], f32, tag="prirep")
            nc.sync.dma_start(out=pri_rep[:],
                                in_=bass.AP(pri_dram, 0, [[0, P], [1, T]]))

            from contextlib import ExitStack
            ps_ctx = ExitStack()
            ps = ps_ctx.enter_context(
                tc.tile_pool(name="psB", bufs=1, space="PSUM"))

            def pair_reduce_bcast(pair_t, tag):
                """[128,2] -> PSUM [128,2]: per-column partition-MAX,
                broadcast to every partition (one PE transpose-matmul +
                one DVE reduce + one PE broadcast-matmul)."""
                tp_ps = ps.tile([2, P], f32, tag="tp2", name=f"tp2{tag}")
                nc.tensor.matmul(out=tp_ps[:], lhsT=pair_t[:], rhs=ID128[:],
                                 start=True, stop=True)
                red2 = sm.tile([2, 1], f32, tag="red2", name=f"red2{tag}")
                nc.vector.tensor_reduce(out=red2[:], in_=tp_ps[:],
                                        axis=mybir.AxisListType.X,
                                        op=mybir.AluOpType.max)
                bc_ps = ps.tile([P, 2], f32, tag="bc2", name=f"bc2{tag}")
                nc.tensor.matmul(out=bc_ps[:],
                                 lhsT=red2[:].to_broadcast([2, P]), rhs=I2[:],
                                 start=True, stop=True)
                return bc_ps

            # ---------------- search init (global min/max) ----------------
            cmax = sm.tile([P, 1], f32, tag="cmax")
            nc.vector.tensor_reduce(out=cmax[:], in_=pri[:],
                                    axis=mybir.AxisListType.X,
                                    op=mybir.AluOpType.max)
            cmin = sm.tile([P, 1], f32, tag="cmin")
            nc.vector.tensor_reduce(out=cmin[:], in_=pri[:],
                                    axis=mybir.AxisListType.X,
                                    op=mybir.AluOpType.min)
            hi = st.tile([P, 1], f32, tag="hi")
            lo = st.tile([P, 1], f32, tag="lo")
            ipair = sm.tile([P, 2], f32, tag="pair", name="ipair")
            nc.vector.tensor_copy(out=ipair[:, 0:1], in_=cmax[:])
            nc.vector.tensor_scalar_mul(ipair[:, 1:2], cmin[:], -1.0)
            ibc = pair_reduce_bcast(ipair, "init")
            nc.vector.tensor_copy(out=hi[:], in_=ibc[:, 0:1])
            nc.vector.tensor_scalar_mul(lo[:], ibc[:, 1:2], -1.0)

            def count_gt(th_t, tag):
                """[P,1] count of pri > th (replicated count, no PE)."""
                cnt = sm.tile([P, 1], f32, tag="cnt", name=f"cnt{tag}")
                nc.vector.tensor_scalar(
                    out=trash[:, :T], in0=pri_rep[:], scalar1=th_t[:, 0:1],
                    scalar2=0.0, op0=mybir.AluOpType.is_gt,
                    op1=mybir.AluOpType.add, accum_out=cnt[:])
                return cnt

            def flags_of(cnt, tag):
                flag = sm.tile([P, 1], i32, tag="bflag", name=f"f{tag}")
                nc.vector.tensor_scalar(
                    out=flag[:], in0=cnt[:], scalar1=float(L), scalar2=None,
                    op0=mybir.AluOpType.is_ge)
                nflag = sm.tile([P, 1], i32, tag="bnflag", name=f"nf{tag}")
                nc.vector.tensor_scalar(
                    out=nflag[:], in0=cnt[:], scalar1=float(L), scalar2=None,
                    op0=mybir.AluOpType.is_lt)
                return flag, nflag

            # ---------------- 128-way multiway search ----------------
            for it in range(N_MULTI):
                d = sm.tile([P, 1], f32, tag="mwd", name=f"d{it}")
                nc.vector.tensor_tensor(out=d[:], in0=hi[:], in1=lo[:],
                                        op=mybir.AluOpType.subtract)
                th = sm.tile([P, 1], f32, tag="mwth", name=f"th{it}")
                nc.vector.scalar_tensor_tensor(
                    out=th[:], in0=d[:], scalar=mstep[:, 0:1], in1=lo[:],
                    op0=mybir.AluOpType.mult, op1=mybir.AluOpType.add)
                cnt = count_gt(th, f"m{it}")
                flag, nflag = flags_of(cnt, f"m{it}")
                negth = sm.tile([P, 1], f32, tag="negth", name=f"nt{it}")
                nc.vector.tensor_scalar_mul(negth[:], th[:], -1.0)
                pair = sm.tile([P, 2], f32, tag="pair", name=f"pair{it}")
                nc.vector.memset(pair[:], -3.0e38)
                nc.vector.copy_predicated(pair[:, 0:1], flag[:], th[:])
                nc.vector.copy_predicated(pair[:, 1:2], nflag[:], negth[:])
                bc = pair_reduce_bcast(pair, f"m{it}")
                nc.vector.tensor_tensor(out=lo[:], in0=lo[:], in1=bc[:, 0:1],
                                        op=mybir.AluOpType.max)
                neghi = sm.tile([P, 1], f32, tag="neghi", name=f"nh{it}")
                nc.vector.tensor_scalar_mul(neghi[:], bc[:, 1:2], -1.0)
                nc.vector.tensor_tensor(out=hi[:], in0=hi[:], in1=neghi[:],
                                        op=mybir.AluOpType.min)

            # ---------------- binary polish to 1 ulp ----------------
            for it in range(N_POLISH):
                mid = sm.tile([P, 1], f32, tag="mid", name=f"mid{it}")
                nc.vector.tensor_scalar(
                    out=mid[:], in0=lo[:], scalar1=hi[:, 0:1], scalar2=0.5,
                    op0=mybir.AluOpType.add, op1=mybir.AluOpType.mult)
                cnt = count_gt(mid, f"p{it}")
                flag, nflag = flags_of(cnt, f"p{it}")
                nc.vector.copy_predicated(lo[:], flag[:], mid[:])
                nc.vector.copy_predicated(hi[:], nflag[:], mid[:])

            # ---------------- selection masks ----------------
            strictM = st.tile([P, F], f32, tag="strictM")
            nc.vector.tensor_scalar(
                out=strictM[:], in0=pri[:], scalar1=hi[:, 0:1], scalar2=None,
                op0=mybir.AluOpType.is_gt)
            scnt = count_gt(hi, "s")
            need = st.tile([P, 1], f32, tag="need")
            nc.vector.tensor_scalar(
                out=need[:], in0=scnt[:], scalar1=-1.0, scalar2=float(L),
                op0=mybir.AluOpType.mult, op1=mybir.AluOpType.add)
            eqgt = sm.tile([P, F], f32, tag="eqgt")
            nc.vector.tensor_scalar(
                out=eqgt[:], in0=pri[:], scalar1=lo[:, 0:1], scalar2=None,
                op0=mybir.AluOpType.is_gt)
            eqM = st.tile([P, F], f32, tag="eqM")
            nc.vector.tensor_tensor(out=eqM[:], in0=eqgt[:], in1=strictM[:],
                                    op=mybir.AluOpType.subtract)

            def cumsum_rm(mask_tile, tag):
                """Inclusive cumsum over t (row-major [128,16]) -> SBUF tile."""
                rsum = sm.tile([P, 1], f32, tag="rsum", name=f"{tag}rsum")
                nc.vector.tensor_reduce(out=rsum[:], in_=mask_tile[:],
                                        axis=mybir.AxisListType.X,
                                        op=mybir.AluOpType.add)
                exo_ps = ps.tile([P, 1], f32, tag="exops", name=f"{tag}exo")
                nc.tensor.matmul(out=exo_ps[:], lhsT=SU128[:], rhs=rsum[:],
                                 start=True, stop=True)
                sa = sm.tile([P, 24], f32, tag="scana", name=f"{tag}sa")
                sb_ = sm.tile([P, 24], f32, tag="scanb", name=f"{tag}sb")
                nc.vector.memset(sa[:], 0.0)
                nc.vector.memset(sb_[:], 0.0)
                nc.vector.tensor_copy(out=sa[:, 8:24], in_=mask_tile[:])
                nc.vector.tensor_tensor(out=sb_[:, 8:24], in0=sa[:, 8:24],
                                        in1=sa[:, 7:23], op=mybir.AluOpType.add)
                nc.vector.tensor_tensor(out=sa[:, 8:24], in0=sb_[:, 8:24],
                                        in1=sb_[:, 6:22], op=mybir.AluOpType.add)
                nc.vector.tensor_tensor(out=sb_[:, 8:24], in0=sa[:, 8:24],
                                        in1=sa[:, 4:20], op=mybir.AluOpType.add)
                exo_sb = sm.tile([P, 1], f32, tag="exosb", name=f"{tag}exosb")
                nc.vector.tensor_copy(out=exo_sb[:], in_=exo_ps[:])
                cum = sm.tile([P, F], f32, tag="cum", name=f"{tag}cum")
                nc.vector.tensor_tensor(out=cum[:], in0=sb_[:, 8:24],
                                        in1=sb_[:, 0:16], op=mybir.AluOpType.add)
                cumt = st.tile([P, F], f32, tag=f"{tag}cumt", name=f"{tag}cumt")
                nc.vector.tensor_scalar(
                    out=cumt[:], in0=cum[:], scalar1=exo_sb[:, 0:1],
                    scalar2=None, op0=mybir.AluOpType.add)
                return cumt

            cum_eq = cumsum_rm(eqM, "a")
            leM = sm.tile([P, F], f32, tag="leM")
            nc.vector.tensor_scalar(
                out=leM[:], in0=cum_eq[:], scalar1=need[:, 0:1],
                scalar2=None, op0=mybir.AluOpType.is_le)
            take = sm.tile([P, F], f32, tag="take")
            nc.vector.tensor_tensor(out=take[:], in0=leM[:], in1=eqM[:],
                                    op=mybir.AluOpType.mult)
            sel = st.tile([P, F], f32, tag="sel")
            nc.vector.tensor_tensor(out=sel[:], in0=strictM[:], in1=take[:],
                                    op=mybir.AluOpType.add)

            possel = cumsum_rm(sel, "b")
            posm1 = sm.tile([P, F], f32, tag="posm1")
            nc.vector.tensor_scalar(
                out=posm1[:], in0=possel[:], scalar1=1.0 + BIG,
                scalar2=None, op0=mybir.AluOpType.subtract)
            selpos = sm.tile([P, F], f32, tag="selpos")
            nc.vector.tensor_tensor(out=selpos[:], in0=posm1[:], in1=sel[:],
                                    op=mybir.AluOpType.mult)
            offf = st.tile([P, F], f32, tag="offf")
            nc.vector.tensor_scalar(
                out=offf[:], in0=selpos[:], scalar1=BIG, scalar2=None,
                op0=mybir.AluOpType.add)
            ps_ctx.close()

            # -------- keep indices via compare-sum against positions ------
            nc.scalar.dma_start(out=pos_dram[0:1, :].rearrange(
                "one (p r) -> one p r", p=P, r=F)[0], in_=offf[:])
            pos_rep = st.tile([P, T], f32, tag="posrep")
            nc.sync.dma_start(out=pos_rep[:],
                              in_=bass.AP(pos_dram, 0, [[0, P], [1, T]]))
            keepf = st.tile([P, 4], f32, tag="keepf")
            for j in range(4):
                nc.vector.scalar_tensor_tensor(
                    out=trash[:, :T], in0=pos_rep[:], scalar=qiota[:, j:j + 1],
                    in1=tio_rep[:], op0=mybir.AluOpType.is_equal,
                    op1=mybir.AluOpType.mult, accum_out=keepf[:, j:j + 1])
            keepi = st.tile([P, 4], i32, tag="keepi")
            nc.vector.tensor_copy(out=keepi[:], in_=keepf[:])
            nc.gpsimd.dma_start(
                out=bass.AP(keep_out, 0, [[4, P], [1, 4]]), in_=keepi[:])

            if debug:
                nc.sync.dma_start(out=dbg_feats[:], in_=feats[:])
                nc.sync.dma_start(out=dbg_pri[:], in_=pri[:])
                nc.sync.dma_start(out=dbg_bounds[:, 0:1], in_=lo[:])
                nc.sync.dma_start(out=dbg_bounds[:, 1:2], in_=hi[:])
                nc.sync.dma_start(out=dbg_sel[:], in_=sel[:])
                nc.sync.dma_start(out=dbg_offf[:], in_=offf[:])
                nc.sync.dma_start(out=dbg_keepf[:], in_=keepf[:])

            # ---------------- gather k/v rows ----------------
            for src, dst in ((k_in, ko_out), (v_in, vo_out)):
                gall = iopool.tile([P, 4 * D], f32, tag="gall")
                for j in range(4):
                    nc.gpsimd.indirect_dma_start(
                        out=gall[:, j * D:(j + 1) * D], out_offset=None,
                        in_=src[:],
                        in_offset=bass.IndirectOffsetOnAxis(
                            ap=keepi[:, j:j + 1], axis=0))
                nc.sync.dma_start(
                    out=bass.AP(dst, 0, [[4 * D, P], [D, 4], [1, D]]),
                    in_=gall[:].rearrange("p (j d) -> p j d", d=D))

    _fix_multi_waits(nc)
    return nc


def kernel(input_pos, k_val, v_val, query, x, attn, W1, b1, W2, b2, **_):
    global last_results
    k_val = np.asarray(k_val, dtype=np.float32)
    v_val = np.asarray(v_val, dtype=np.float32)
    query = np.asarray(query, dtype=np.float32)
    x = np.asarray(x, dtype=np.float32)
    attn = np.asarray(attn, dtype=np.float32)
    W1 = np.asarray(W1, dtype=np.float32)
    b1 = np.asarray(b1, dtype=np.float32)
    W2 = np.asarray(W2, dtype=np.float32)
    b2 = np.asarray(b2, dtype=np.float32)

    nc = _build()
    in_maps = []
    for h in range(W):
        in_maps.append({
            "attn": np.ascontiguousarray(attn[0, h]),
            "k": np.ascontiguousarray(k_val[0, h]),
            "v": np.ascontiguousarray(v_val[0, h]),
            "q": np.ascontiguousarray(query[0, G * h:G * (h + 1)]),
            "xs": np.ascontiguousarray(x[0, TS * h:TS * (h + 1)]),
            "w1r": np.broadcast_to(
                W1[h].T.reshape(1, HID * NF), (P, HID * NF)).copy(),
            "b1r": np.broadcast_to(b1[h][None, :], (P, HID)).copy(),
            "w2r": np.broadcast_to(W2[h][:, 0][None, :], (P, HID)).copy(),
            "b2r": np.broadcast_to(b2[h][None, :], (P, 1)).copy(),
        })
    res = run_bass_kernel_spmd(nc, in_maps, list(range(W)))
    last_results = res

    keep = np.stack([res.results[h]["keep"][:, 0] for h in range(W)]).astype(np.int32)
    k_out = np.stack([res.results[h]["ko"] for h in range(W)])[None]
    v_out = np.stack([res.results[h]["vo"] for h in range(W)])[None]
    return keep, k_out, v_out


# revision 25
# speedup vs baseline: 1.0899x; 1.0899x over previous
"""Trainium2 Bass kernel for nn_PromptCompressorLightweight (iteration 2).

Head-parallel SPMD over 8 NeuronCores (1 KV head per core), row-major
t-layout (t = p*16 + r):
  - attn weighted column-sum on PE (w stationary, attn streaming)
  - k/v/q norms from single 1MB contiguous loads, square+reduce on DVE
  - x-norm sequence-sharded across cores + AllGather
  - tiny per-head MLP on DVE
  - exact top-512: f32 bisection + tie fill; positions via strict-upper
    matmul (partition scan) + shifted-add free scan; keep indices
    recovered by compare-sum against broadcast position array (no
    scatter); k/v rows fetched with 8 indirect gathers.
"""
import os

import numpy as np

import concourse.bass as bass
import concourse.mybir as mybir
import concourse.tile as tile
from concourse.bass_utils import run_bass_kernel_spmd
from concourse.masks import make_identity, make_upper_triangular

B, H, HQ, T, D, M = 1, 8, 32, 2048, 128, 4096
G = HQ // H
NF, HID = 5, 10
L = 512
P, F = 128, 16       # row-major: t = p*16 + r
W = 8
TS = T // W
N_MULTI = 5          # 128-way threshold-search rounds
N_POLISH = 3         # binary polish rounds (to 1 ulp)
BIG = 8192.0

f32 = mybir.dt.float32
i32 = mybir.dt.int32

last_results = None


def _fix_multi_waits(nc):
    """This container's walrus rejects >1 sync wait per instruction. Move
    extra waits onto nop carriers inserted just before, on the same engine."""
    n_split = 0
    for f in nc.m.functions:
        for bb in f.blocks:
            insts = bb.instructions
            i = 0
            while i < len(insts):
                inst = insts[i]
                si = getattr(inst, "sync_info", None)
                waits = list(si.on_wait) if si is not None and si.on_wait else []
                if len(waits) > 1:
                    inst.sync_info = mybir.SyncInfo(
                        on_wait=waits[-1:],
                        on_update=list(si.on_update) if si.on_update else [],
                    )
                    for k, wt in enumerate(waits[:-1]):
                        nop = mybir.InstNoOp(
                            name=f"waitfix-{n_split}-{k}", ins=[], outs=[])
                        nop.engine = inst.engine
                        nop.sync_info = mybir.SyncInfo(on_wait=[wt], on_update=[])
                        nc.register_instruction(nop, overwrite=True)
                        insts.insert(i, nop)
                        i += 1
                    n_split += 1
                i += 1
    return n_split


def _bcast_mid(ap, n_mid):
    """[P, k] AP -> [P, n_mid, k] with a step-0 middle dim."""
    return bass.AP(ap.tensor, ap.offset, [ap.ap[0], [0, n_mid], ap.ap[1]])


def _build():
    nc = bass.Bass("TRN2", num_devices=W)

    attn_in = nc.dram_tensor("attn", [T, T], f32, kind="ExternalInput")
    k_in = nc.dram_tensor("k", [T, D], f32, kind="ExternalInput")
    v_in = nc.dram_tensor("v", [T, D], f32, kind="ExternalInput")
    q_in = nc.dram_tensor("q", [G, T, D], f32, kind="ExternalInput")
    x_in = nc.dram_tensor("xs", [TS, M], f32, kind="ExternalInput")
    w1r_in = nc.dram_tensor("w1r", [P, HID * NF], f32, kind="ExternalInput")
    b1r_in = nc.dram_tensor("b1r", [P, HID], f32, kind="ExternalInput")
    w2r_in = nc.dram_tensor("w2r", [P, HID], f32, kind="ExternalInput")
    b2r_in = nc.dram_tensor("b2r", [P, 1], f32, kind="ExternalInput")

    debug = bool(int(os.environ.get("KDEBUG", "0")))
    keep_out = nc.dram_tensor("keep", [L, 1], i32, kind="ExternalOutput")
    ko_out = nc.dram_tensor("ko", [L, D], f32, kind="ExternalOutput")
    vo_out = nc.dram_tensor("vo", [L, D], f32, kind="ExternalOutput")
    if debug:
        dbg_feats = nc.dram_tensor("dbg_feats", [P, F * NF], f32, kind="ExternalOutput")
        dbg_pri = nc.dram_tensor("dbg_pri", [P, F], f32, kind="ExternalOutput")
        dbg_bounds = nc.dram_tensor("dbg_bounds", [P, 2], f32, kind="ExternalOutput")
        dbg_sel = nc.dram_tensor("dbg_sel", [P, F], f32, kind="ExternalOutput")
        dbg_offf = nc.dram_tensor("dbg_offf", [P, F], f32, kind="ExternalOutput")
        dbg_keepf = nc.dram_tensor("dbg_keepf", [P, 4], f32, kind="ExternalOutput")

    as_dram = nc.dram_tensor("as_dram", [1, T], f32)
    pri_dram = nc.dram_tensor("pri_dram", [1, T], f32)
    pos_dram = nc.dram_tensor("pos_dram", [1, T], f32)
    cc_in = nc.dram_tensor("cc_in", [1, TS], f32)
    cc_out = nc.dram_tensor("cc_out", [1, T], f32, addr_space="Shared")

    # inline consts
    wrow_np = (np.arange(F)[None, :] * P + np.arange(P)[:, None] + 1).astype(np.float32)
    wrow_d = nc.inline_tensor(wrow_np, "wrow")   # attn row weights, i = c*128+p
    recipd_d = nc.inline_tensor(
        (1.0 / np.arange(T, 0, -1, dtype=np.float32))[None, :], "recipd")
    tio_d = nc.inline_tensor(np.arange(T, dtype=np.float32)[None, :], "tiorow")
    qiota_np = (np.arange(P)[:, None] * 4 + np.arange(4)[None, :]).astype(np.float32)
    qiota_d = nc.inline_tensor(qiota_np, "qiota")  # q = p*4 + j
    mstep_d = nc.inline_tensor(
        ((np.arange(P) + 1.0) / 129.0).astype(np.float32)[:, None], "mstep")

    with tile.TileContext(nc) as tc:
        with tc.tile_pool(name="const", bufs=1) as cpool, \
             tc.tile_pool(name="attn", bufs=3) as apool, \
             tc.tile_pool(name="big", bufs=1) as bpool, \
             tc.tile_pool(name="xp", bufs=1) as xpool, \
             tc.tile_pool(name="io", bufs=4) as iopool, \
             tc.tile_pool(name="sm", bufs=2) as sm, \
             tc.tile_pool(name="st", bufs=1) as st:

            # ---------------- constants ----------------
            SU128 = cpool.tile([P, P], f32, tag="su128")
            make_upper_triangular(nc, SU128[:], val=1.0, diag=False)
            ID128 = cpool.tile([P, P], f32, tag="id128")
            make_identity(nc, ID128[:])
            I2 = cpool.tile([2, 2], f32, tag="i2")
            make_identity(nc, I2[:])
            ones128 = cpool.tile([P, P], f32, tag="ones128")
            nc.vector.memset(ones128[:], 1.0)
            onesrow = cpool.tile([1, P], f32, tag="onesrow")
            nc.vector.memset(onesrow[:], 1.0)
            wrow = cpool.tile([P, F], f32, tag="wrow")
            nc.scalar.dma_start(out=wrow[:], in_=wrow_d[:])
            recipd = cpool.tile([1, T], f32, tag="recipd")
            nc.scalar.dma_start(out=recipd[:], in_=recipd_d[:])
            qiota = cpool.tile([P, 4], f32, tag="qiota")
            nc.scalar.dma_start(out=qiota[:], in_=qiota_d[:])
            mstep = cpool.tile([P, 1], f32, tag="mstep")
            nc.scalar.dma_start(out=mstep[:], in_=mstep_d[:])
            tio_rep = cpool.tile([P, T], f32, tag="tiorep")
            nc.scalar.dma_start(out=tio_rep[:],
                              in_=bass.AP(tio_d, 0, [[0, P], [1, T]]))
            w1r = cpool.tile([P, HID * NF], f32, tag="w1r")
            nc.scalar.dma_start(out=w1r[:], in_=w1r_in[:])
            b1r = cpool.tile([P, HID], f32, tag="b1r")
            nc.scalar.dma_start(out=b1r[:], in_=b1r_in[:])
            w2r = cpool.tile([P, HID], f32, tag="w2r")
            nc.scalar.dma_start(out=w2r[:], in_=w2r_in[:])
            b2r = cpool.tile([P, 1], f32, tag="b2r")
            nc.scalar.dma_start(out=b2r[:], in_=b2r_in[:])

            feats = st.tile([P, F * NF], f32, tag="feats")  # [128,16,5]
            feats3 = feats[:].rearrange("p (c f) -> p c f", f=NF)
            trash = st.tile([P, M], f32, tag="bigtrash")  # shared scratch
            trash2 = trash[:, :F * D]
            trash3 = trash2.rearrange("p (r d) -> p r d", d=D)

            # ---------------- x norm (sharded) + allgather ----------------
            hp_ctx = tc.high_priority()
            hp_ctx.__enter__()
            xn = st.tile([P, 2], f32, tag="xn")
            for c in range(2):
                xc = xpool.tile([P, M], f32, tag="xc", bufs=2)
                nc.sync.dma_start(out=xc[:], in_=x_in[c * P:(c + 1) * P, :])
                nc.vector.scalar_tensor_tensor(
                    out=trash[:], in0=xc[:], scalar=0.0, in1=xc[:],
                    op0=mybir.AluOpType.add, op1=mybir.AluOpType.mult,
                    accum_out=xn[:, c:c + 1])
            xnr = st.tile([P, 2], f32, tag="xnr")
            nc.scalar.sqrt(xnr[:], xn[:])
            for c in range(2):
                nc.gpsimd.dma_start(out=cc_in[0, c * P:(c + 1) * P],
                                    in_=xnr[:, c:c + 1])
            nc.gpsimd.collective_compute(
                "AllGather", mybir.AluOpType.bypass,
                replica_groups=[list(range(W))],
                ins=[cc_in[:]], outs=[cc_out[:]])
            cc_rm = cc_out[:].rearrange("one (p r) -> one p r", p=P, r=F)
            nc.scalar.dma_start(out=feats3[:, :, 4], in_=cc_rm[0])
            hp_ctx.__exit__(None, None, None)

            # ---------------- attn weighted colsum ----------------
            with tc.tile_pool(name="psA", bufs=1, space="PSUM") as psA:
                as_ps = [psA.tile([1, 512], f32, tag=f"asps{n}",
                                  name=f"asps{n}") for n in range(4)]
                for c in range(F):
                    at = apool.tile([P, T], f32, tag="at")
                    nc.sync.dma_start(out=at[:],
                                      in_=attn_in[c * P:(c + 1) * P, :])
                    for n in range(4):
                        nc.tensor.matmul(
                            out=as_ps[n][:], lhsT=wrow[:, c:c + 1],
                            rhs=at[:, n * 512:(n + 1) * 512],
                            start=(c == 0), stop=(c == F - 1))
                as_row = st.tile([1, T], f32, tag="asrow")
                for n in range(4):
                    nc.vector.tensor_tensor(
                        out=as_row[:, n * 512:(n + 1) * 512], in0=as_ps[n][:],
                        in1=recipd[:, n * 512:(n + 1) * 512],
                        op=mybir.AluOpType.mult)
            nc.scalar.dma_start(out=as_dram[:], in_=as_row[:])
            as_rm = as_dram[:].rearrange("one (p r) -> one p r", p=P, r=F)
            nc.scalar.dma_start(out=feats3[:, :, 0], in_=as_rm[0])

            # ---------------- k/v/q norms (1MB contiguous loads) ----------
            for name_, src, fi in (("kn", k_in, 1), ("vn", v_in, 2)):
                kb = bpool.tile([P, F * D], f32, tag="kb", bufs=2)
                kb3 = kb[:].rearrange("p (r d) -> p r d", d=D)
                nc.sync.dma_start(
                    out=kb3, in_=src[:].rearrange("(p r) d -> p r d", p=P, r=F))
                nc.vector.tensor_tensor(out=trash2, in0=kb[:], in1=kb[:],
                                        op=mybir.AluOpType.mult)
                n2 = st.tile([P, F], f32, tag=f"{name_}2", name=f"{name_}2")
                nc.vector.tensor_reduce(out=n2[:], in_=trash3,
                                        axis=mybir.AxisListType.X,
                                        op=mybir.AluOpType.add)
                nc.scalar.sqrt(feats3[:, :, fi], n2[:])
            qs = []
            for g in range(G):
                qt = bpool.tile([P, F * D], f32, tag=f"qt{g}", name=f"qt{g}")
                nc.sync.dma_start(
                    out=qt[:].rearrange("p (r d) -> p r d", d=D),
                    in_=q_in[g].rearrange("(p r) d -> p r d", p=P, r=F))
                qs.append(qt)
            nc.vector.tensor_tensor(out=qs[0][:], in0=qs[0][:], in1=qs[1][:],
                                    op=mybir.AluOpType.add)
            nc.vector.tensor_tensor(out=qs[2][:], in0=qs[2][:], in1=qs[3][:],
                                    op=mybir.AluOpType.add)
            nc.vector.tensor_tensor(out=qs[0][:], in0=qs[0][:], in1=qs[2][:],
                                    op=mybir.AluOpType.add)
            nc.vector.tensor_tensor(out=trash2, in0=qs[0][:], in1=qs[0][:],
                                    op=mybir.AluOpType.mult)
            qn2 = st.tile([P, F], f32, tag="qn2")
            nc.vector.tensor_reduce(out=qn2[:], in_=trash3,
                                    axis=mybir.AxisListType.X,
                                    op=mybir.AluOpType.add)
            nc.scalar.activation(feats3[:, :, 3], qn2[:],
                                 mybir.ActivationFunctionType.Sqrt,
                                 scale=1.0 / (G * G))

            # ---------------- MLP ----------------
            h3d = st.tile([P, F * HID], f32, tag="h3d")
            h3 = h3d[:].rearrange("p (c j) -> p c j", j=HID)
            mul5 = sm.tile([P, F * NF], f32, tag="mul5")
            mul5_3 = mul5[:].rearrange("p (c f) -> p c f", f=NF)
            for j in range(HID):
                w1j = _bcast_mid(w1r[:, j * NF:(j + 1) * NF], F)
                nc.vector.tensor_tensor(out=mul5_3, in0=feats3, in1=w1j,
                                        op=mybir.AluOpType.mult)
                nc.vector.tensor_reduce(
                    out=h3[:, :, j], in_=mul5_3, axis=mybir.AxisListType.X,
                    op=mybir.AluOpType.add)
            b1b = _bcast_mid(b1r[:], F)
            nc.vector.tensor_tensor(out=h3, in0=h3, in1=b1b,
                                    op=mybir.AluOpType.add)
            nc.scalar.activation(h3d[:], h3d[:],
                                 mybir.ActivationFunctionType.Relu)
            mul10 = sm.tile([P, F * HID], f32, tag="mul10")
            mul10_3 = mul10[:].rearrange("p (c j) -> p c j", j=HID)
            w2b = _bcast_mid(w2r[:], F)
            nc.vector.tensor_tensor(out=mul10_3, in0=h3, in1=w2b,
                                    op=mybir.AluOpType.mult)
            pri = st.tile([P, F], f32, tag="pri")
            nc.vector.tensor_reduce(out=pri[:], in_=mul10_3,
                                    axis=mybir.AxisListType.X,
                                    op=mybir.AluOpType.add)
            nc.vector.tensor_scalar(out=pri[:], in0=pri[:],
                                    scalar1=b2r[:, 0:1], scalar2=None,
                                    op0=mybir.AluOpType.add)
            # replicate priorities to [128, T] (each partition = full array)
            nc.scalar.dma_start(out=pri_dram[0:1, :].rearrange(
                "one (p r) -> one p r", p=P, r=F)[0], in_=pri[:])
            pri_rep = st.tile([P, T], f32, tag="prirep")
            nc.sync.dma_start(out=pri_rep[:],
                                in_=bass.AP(pri_dram, 0, [[0, P], [1, T]]))

            from contextlib import ExitStack
            ps_ctx = ExitStack()
            ps = ps_ctx.enter_context(
                tc.tile_pool(name="psB", bufs=1, space="PSUM"))

            def pair_reduce_bcast(pair_t, tag):
                """[128,2] -> PSUM [128,2]: per-column partition-MAX,
                broadcast to every partition (one PE transpose-matmul +
                one DVE reduce + one PE broadcast-matmul)."""
                tp_ps = ps.tile([2, P], f32, tag="tp2", name=f"tp2{tag}")
                nc.tensor.matmul(out=tp_ps[:], lhsT=pair_t[:], rhs=ID128[:],
                                 start=True, stop=True)
                red2 = sm.tile([2, 1], f32, tag="red2", name=f"red2{tag}")
                nc.vector.tensor_reduce(out=red2[:], in_=tp_ps[:],
                                        axis=mybir.AxisListType.X,
                                        op=mybir.AluOpType.max)
                bc_ps = ps.tile([P, 2], f32, tag="bc2", name=f"bc2{tag}")
                nc.tensor.matmul(out=bc_ps[:],
                                 lhsT=red2[:].to_broadcast([2, P]), rhs=I2[:],
                                 start=True, stop=True)
                return bc_ps

            # ---------------- search init (global min/max) ----------------
            cmax = sm.tile([P, 1], f32, tag="cmax")
            nc.vector.tensor_reduce(out=cmax[:], in_=pri[:],
                                    axis=mybir.AxisListType.X,
                                    op=mybir.AluOpType.max)
            cmin = sm.tile([P, 1], f32, tag="cmin")
            nc.vector.tensor_reduce(out=cmin[:], in_=pri[:],
                                    axis=mybir.AxisListType.X,
                                    op=mybir.AluOpType.min)
            hi = st.tile([P, 1], f32, tag="hi")
            lo = st.tile([P, 1], f32, tag="lo")
            ipair = sm.tile([P, 2], f32, tag="pair", name="ipair")
            nc.vector.tensor_copy(out=ipair[:, 0:1], in_=cmax[:])
            nc.vector.tensor_scalar_mul(ipair[:, 1:2], cmin[:], -1.0)
            ibc = pair_reduce_bcast(ipair, "init")
            nc.vector.tensor_copy(out=hi[:], in_=ibc[:, 0:1])
            nc.vector.tensor_scalar_mul(lo[:], ibc[:, 1:2], -1.0)

            def count_gt(th_t, tag):
                """[P,1] count of pri > th (replicated count, no PE)."""
                cnt = sm.tile([P, 1], f32, tag="cnt", name=f"cnt{tag}")
                nc.vector.tensor_scalar(
                    out=trash[:, :T], in0=pri_rep[:], scalar1=th_t[:, 0:1],
                    scalar2=0.0, op0=mybir.AluOpType.is_gt,
                    op1=mybir.AluOpType.add, accum_out=cnt[:])
                return cnt

            def flags_of(cnt, tag):
                flag = sm.tile([P, 1], i32, tag="bflag", name=f"f{tag}")
                nc.vector.tensor_scalar(
                    out=flag[:], in0=cnt[:], scalar1=float(L), scalar2=None,
                    op0=mybir.AluOpType.is_ge)
                nflag = sm.tile([P, 1], i32, tag="bnflag", name=f"nf{tag}")
                nc.vector.tensor_scalar(
                    out=nflag[:], in0=cnt[:], scalar1=float(L), scalar2=None,
                    op0=mybir.AluOpType.is_lt)
                return flag, nflag

            # ---------------- 128-way multiway search ----------------
            for it in range(N_MULTI):
                d = sm.tile([P, 1], f32, tag="mwd", name=f"d{it}")
                nc.vector.tensor_tensor(out=d[:], in0=hi[:], in1=lo[:],
                                        op=mybir.AluOpType.subtract)
                th = sm.tile([P, 1], f32, tag="mwth", name=f"th{it}")
                nc.vector.scalar_tensor_tensor(
                    out=th[:], in0=d[:], scalar=mstep[:, 0:1], in1=lo[:],
                    op0=mybir.AluOpType.mult, op1=mybir.AluOpType.add)
                cnt = count_gt(th, f"m{it}")
                flag, nflag = flags_of(cnt, f"m{it}")
                negth = sm.tile([P, 1], f32, tag="negth", name=f"nt{it}")
                nc.vector.tensor_scalar_mul(negth[:], th[:], -1.0)
                pair = sm.tile([P, 2], f32, tag="pair", name=f"pair{it}")
                nc.vector.memset(pair[:], -3.0e38)
                nc.vector.copy_predicated(pair[:, 0:1], flag[:], th[:])
                nc.vector.copy_predicated(pair[:, 1:2], nflag[:], negth[:])
                bc = pair_reduce_bcast(pair, f"m{it}")
                nc.vector.tensor_tensor(out=lo[:], in0=lo[:], in1=bc[:, 0:1],
                                        op=mybir.AluOpType.max)
                neghi = sm.tile([P, 1], f32, tag="neghi", name=f"nh{it}")
                nc.vector.tensor_scalar_mul(neghi[:], bc[:, 1:2], -1.0)
                nc.vector.tensor_tensor(out=hi[:], in0=hi[:], in1=neghi[:],
                                        op=mybir.AluOpType.min)

            # ---------------- binary polish to 1 ulp ----------------
            for it in range(N_POLISH):
                mid = sm.tile([P, 1], f32, tag="mid", name=f"mid{it}")
                nc.vector.tensor_scalar(
                    out=mid[:], in0=lo[:], scalar1=hi[:, 0:1], scalar2=0.5,
                    op0=mybir.AluOpType.add, op1=mybir.AluOpType.mult)
                cnt = count_gt(mid, f"p{it}")
                flag, nflag = flags_of(cnt, f"p{it}")
                nc.vector.copy_predicated(lo[:], flag[:], mid[:])
                nc.vector.copy_predicated(hi[:], nflag[:], mid[:])

            # ---------------- selection masks ----------------
            strictM = st.tile([P, F], f32, tag="strictM")
            nc.vector.tensor_scalar(
                out=strictM[:], in0=pri[:], scalar1=hi[:, 0:1], scalar2=None,
                op0=mybir.AluOpType.is_gt)
            scnt = count_gt(hi, "s")
            need = st.tile([P, 1], f32, tag="need")
            nc.vector.tensor_scalar(
                out=need[:], in0=scnt[:], scalar1=-1.0, scalar2=float(L),
                op0=mybir.AluOpType.mult, op1=mybir.AluOpType.add)
            eqgt = sm.tile([P, F], f32, tag="eqgt")
            nc.vector.tensor_scalar(
                out=eqgt[:], in0=pri[:], scalar1=lo[:, 0:1], scalar2=None,
                op0=mybir.AluOpType.is_gt)
            eqM = st.tile([P, F], f32, tag="eqM")
            nc.vector.tensor_tensor(out=eqM[:], in0=eqgt[:], in1=strictM[:],
                                    op=mybir.AluOpType.subtract)

            def cumsum_rm(mask_tile, tag):
                """Inclusive cumsum over t (row-major [128,16]) -> SBUF tile."""
                rsum = sm.tile([P, 1], f32, tag="rsum", name=f"{tag}rsum")
                nc.vector.tensor_reduce(out=rsum[:], in_=mask_tile[:],
                                        axis=mybir.AxisListType.X,
                                        op=mybir.AluOpType.add)
                exo_ps = ps.tile([P, 1], f32, tag="exops", name=f"{tag}exo")
                nc.tensor.matmul(out=exo_ps[:], lhsT=SU128[:], rhs=rsum[:],
                                 start=True, stop=True)
                sa = sm.tile([P, 24], f32, tag="scana", name=f"{tag}sa")
                sb_ = sm.tile([P, 24], f32, tag="scanb", name=f"{tag}sb")
                nc.vector.memset(sa[:], 0.0)
                nc.vector.memset(sb_[:], 0.0)
                nc.vector.tensor_copy(out=sa[:, 8:24], in_=mask_tile[:])
                nc.vector.tensor_tensor(out=sb_[:, 8:24], in0=sa[:, 8:24],
                                        in1=sa[:, 7:23], op=mybir.AluOpType.add)
                nc.vector.tensor_tensor(out=sa[:, 8:24], in0=sb_[:, 8:24],
                                        in1=sb_[:, 6:22], op=mybir.AluOpType.add)
                nc.vector.tensor_tensor(out=sb_[:, 8:24], in0=sa[:, 8:24],
                                        in1=sa[:, 4:20], op=mybir.AluOpType.add)
                exo_sb = sm.tile([P, 1], f32, tag="exosb", name=f"{tag}exosb")
                nc.vector.tensor_copy(out=exo_sb[:], in_=exo_ps[:])
                cum = sm.tile([P, F], f32, tag="cum", name=f"{tag}cum")
                nc.vector.tensor_tensor(out=cum[:], in0=sb_[:, 8:24],
                                        in1=sb_[:, 0:16], op=mybir.AluOpType.add)
                cumt = st.tile([P, F], f32, tag=f"{tag}cumt", name=f"{tag}cumt")
                nc.vector.tensor_scalar(
                    out=cumt[:], in0=cum[:], scalar1=exo_sb[:, 0:1],
                    scalar2=None, op0=mybir.AluOpType.add)
                return cumt

            cum_eq = cumsum_rm(eqM, "a")
            leM = sm.tile([P, F], f32, tag="leM")
            nc.vector.tensor_scalar(
                out=leM[:], in0=cum_eq[:], scalar1=need[:, 0:1],
                scalar2=None, op0=mybir.AluOpType.is_le)
            take = sm.tile([P, F], f32, tag="take")
            nc.vector.tensor_tensor(out=take[:], in0=leM[:], in1=eqM[:],
                                    op=mybir.AluOpType.mult)
            sel = st.tile([P, F], f32, tag="sel")
            nc.vector.tensor_tensor(out=sel[:], in0=strictM[:], in1=take[:],
                                    op=mybir.AluOpType.add)

            possel = cumsum_rm(sel, "b")
            posm1 = sm.tile([P, F], f32, tag="posm1")
            nc.vector.tensor_scalar(
                out=posm1[:], in0=possel[:], scalar1=1.0 + BIG,
                scalar2=None, op0=mybir.AluOpType.subtract)
            selpos = sm.tile([P, F], f32, tag="selpos")
            nc.vector.tensor_tensor(out=selpos[:], in0=posm1[:], in1=sel[:],
                                    op=mybir.AluOpType.mult)
            offf = st.tile([P, F], f32, tag="offf")
            nc.vector.tensor_scalar(
                out=offf[:], in0=selpos[:], scalar1=BIG, scalar2=None,
                op0=mybir.AluOpType.add)
            ps_ctx.close()

            # -------- keep indices via compare-sum against positions ------
            nc.scalar.dma_start(out=pos_dram[0:1, :].rearrange(
                "one (p r) -> one p r", p=P, r=F)[0], in_=offf[:])
            pos_rep = st.tile([P, T], f32, tag="posrep")
            nc.sync.dma_start(out=pos_rep[:],
                              in_=bass.AP(pos_dram, 0, [[0, P], [1, T]]))
            keepf = st.tile([P, 4], f32, tag="keepf")
            for j in range(4):
                nc.vector.scalar_tensor_tensor(
                    out=trash[:, :T], in0=pos_rep[:], scalar=qiota[:, j:j + 1],
                    in1=tio_rep[:], op0=mybir.AluOpType.is_equal,
                    op1=mybir.AluOpType.mult, accum_out=keepf[:, j:j + 1])
            keepi = st.tile([P, 4], i32, tag="keepi")
            nc.vector.tensor_copy(out=keepi[:], in_=keepf[:])
            nc.gpsimd.dma_start(
                out=bass.AP(keep_out, 0, [[4, P], [1, 4]]), in_=keepi[:])

            if debug:
                nc.sync.dma_start(out=dbg_feats[:], in_=feats[:])
                nc.sync.dma_start(out=dbg_pri[:], in_=pri[:])
                nc.sync.dma_start(out=dbg_bounds[:, 0:1], in_=lo[:])
                nc.sync.dma_start(out=dbg_bounds[:, 1:2], in_=hi[:])
                nc.sync.dma_start(out=dbg_sel[:], in_=sel[:])
                nc.sync.dma_start(out=dbg_offf[:], in_=offf[:])
                nc.sync.dma_start(out=dbg_keepf[:], in_=keepf[:])

            # ---------------- gather k/v rows ----------------
            for src, dst in ((k_in, ko_out), (v_in, vo_out)):
                gall = iopool.tile([P, 4 * D], f32, tag="gall")
                for j in range(4):
                    nc.gpsimd.indirect_dma_start(
                        out=gall[:, j * D:(j + 1) * D], out_offset=None,
                        in_=src[:],
                        in_offset=bass.IndirectOffsetOnAxis(
                            ap=keepi[:, j:j + 1], axis=0))
                nc.sync.dma_start(
                    out=bass.AP(dst, 0, [[4 * D, P], [D, 4], [1, D]]),
                    in_=gall[:].rearrange("p (j d) -> p j d", d=D))

    _fix_multi_waits(nc)
    return nc


def kernel(input_pos, k_val, v_val, query, x, attn, W1, b1, W2, b2, **_):
    global last_results
    k_val = np.asarray(k_val, dtype=np.float32)
    v_val = np.asarray(v_val, dtype=np.float32)
    query = np.asarray(query, dtype=np.float32)
    x = np.asarray(x, dtype=np.float32)
    attn = np.asarray(attn, dtype=np.float32)
    W1 = np.asarray(W1, dtype=np.float32)
    b1 = np.asarray(b1, dtype=np.float32)
    W2 = np.asarray(W2, dtype=np.float32)
    b2 = np.asarray(b2, dtype=np.float32)

    nc = _build()
    in_maps = []
    for h in range(W):
        in_maps.append({
            "attn": np.ascontiguousarray(attn[0, h]),
            "k": np.ascontiguousarray(k_val[0, h]),
            "v": np.ascontiguousarray(v_val[0, h]),
            "q": np.ascontiguousarray(query[0, G * h:G * (h + 1)]),
            "xs": np.ascontiguousarray(x[0, TS * h:TS * (h + 1)]),
            "w1r": np.broadcast_to(
                W1[h].T.reshape(1, HID * NF), (P, HID * NF)).copy(),
            "b1r": np.broadcast_to(b1[h][None, :], (P, HID)).copy(),
            "w2r": np.broadcast_to(W2[h][:, 0][None, :], (P, HID)).copy(),
            "b2r": np.broadcast_to(b2[h][None, :], (P, 1)).copy(),
        })
    res = run_bass_kernel_spmd(nc, in_maps, list(range(W)))
    last_results = res

    keep = np.stack([res.results[h]["keep"][:, 0] for h in range(W)]).astype(np.int32)
    k_out = np.stack([res.results[h]["ko"] for h in range(W)])[None]
    v_out = np.stack([res.results[h]["vo"] for h in range(W)])[None]
    return keep, k_out, v_out


# revision 34
# speedup vs baseline: 1.1069x; 1.0156x over previous
"""Trainium2 Bass kernel for nn_PromptCompressorLightweight (iteration 2).

Head-parallel SPMD over 8 NeuronCores (1 KV head per core), row-major
t-layout (t = p*16 + r):
  - attn weighted column-sum on PE (w stationary, attn streaming)
  - k/v/q norms from single 1MB contiguous loads, square+reduce on DVE
  - x-norm sequence-sharded across cores + AllGather
  - tiny per-head MLP on DVE
  - exact top-512: f32 bisection + tie fill; positions via strict-upper
    matmul (partition scan) + shifted-add free scan; keep indices
    recovered by compare-sum against broadcast position array (no
    scatter); k/v rows fetched with 8 indirect gathers.
"""
import os

import numpy as np

import concourse.bass as bass
import concourse.mybir as mybir
import concourse.tile as tile
from concourse.bass_utils import run_bass_kernel_spmd
from concourse.masks import make_identity, make_upper_triangular

B, H, HQ, T, D, M = 1, 8, 32, 2048, 128, 4096
G = HQ // H
NF, HID = 5, 10
L = 512
P, F = 128, 16       # row-major: t = p*16 + r
W = 8
TS = T // W
N_MULTI = 5          # 128-way threshold-search rounds
N_POLISH = 2         # binary polish rounds (to 1 ulp)
BIG = 8192.0

f32 = mybir.dt.float32
f16 = mybir.dt.float16
i32 = mybir.dt.int32

last_results = None


def _fix_multi_waits(nc):
    """This container's walrus rejects >1 sync wait per instruction. Move
    extra waits onto nop carriers inserted just before, on the same engine."""
    n_split = 0
    for f in nc.m.functions:
        for bb in f.blocks:
            insts = bb.instructions
            i = 0
            while i < len(insts):
                inst = insts[i]
                si = getattr(inst, "sync_info", None)
                waits = list(si.on_wait) if si is not None and si.on_wait else []
                if len(waits) > 1:
                    inst.sync_info = mybir.SyncInfo(
                        on_wait=waits[-1:],
                        on_update=list(si.on_update) if si.on_update else [],
                    )
                    for k, wt in enumerate(waits[:-1]):
                        nop = mybir.InstNoOp(
                            name=f"waitfix-{n_split}-{k}", ins=[], outs=[])
                        nop.engine = inst.engine
                        nop.sync_info = mybir.SyncInfo(on_wait=[wt], on_update=[])
                        nc.register_instruction(nop, overwrite=True)
                        insts.insert(i, nop)
                        i += 1
                    n_split += 1
                i += 1
    return n_split


def _bcast_mid(ap, n_mid):
    """[P, k] AP -> [P, n_mid, k] with a step-0 middle dim."""
    return bass.AP(ap.tensor, ap.offset, [ap.ap[0], [0, n_mid], ap.ap[1]])


def _build():
    nc = bass.Bass("TRN2", num_devices=W)

    attn_in = nc.dram_tensor("attn", [T, T], f32, kind="ExternalInput")
    k_in = nc.dram_tensor("k", [T, D], f32, kind="ExternalInput")
    v_in = nc.dram_tensor("v", [T, D], f32, kind="ExternalInput")
    q_in = nc.dram_tensor("q", [G, T, D], f32, kind="ExternalInput")
    x_in = nc.dram_tensor("xs", [TS, M], f32, kind="ExternalInput")
    w1r_in = nc.dram_tensor("w1r", [P, HID * NF], f32, kind="ExternalInput")
    b1r_in = nc.dram_tensor("b1r", [P, HID], f32, kind="ExternalInput")
    w2r_in = nc.dram_tensor("w2r", [P, HID], f32, kind="ExternalInput")
    b2r_in = nc.dram_tensor("b2r", [P, 1], f32, kind="ExternalInput")

    debug = bool(int(os.environ.get("KDEBUG", "0")))
    keep_out = nc.dram_tensor("keep", [L, 1], i32, kind="ExternalOutput")
    ko_out = nc.dram_tensor("ko", [L, D], f32, kind="ExternalOutput")
    vo_out = nc.dram_tensor("vo", [L, D], f32, kind="ExternalOutput")
    if debug:
        dbg_feats = nc.dram_tensor("dbg_feats", [P, F * NF], f32, kind="ExternalOutput")
        dbg_pri = nc.dram_tensor("dbg_pri", [P, F], f32, kind="ExternalOutput")
        dbg_bounds = nc.dram_tensor("dbg_bounds", [P, 2], f32, kind="ExternalOutput")
        dbg_sel = nc.dram_tensor("dbg_sel", [P, F], f32, kind="ExternalOutput")
        dbg_offf = nc.dram_tensor("dbg_offf", [P, F], f32, kind="ExternalOutput")
        dbg_keepf = nc.dram_tensor("dbg_keepf", [P, 4], f32, kind="ExternalOutput")

    as_dram = nc.dram_tensor("as_dram", [1, T], f32)
    pri_dram = nc.dram_tensor("pri_dram", [1, T], f32)
    pos_dram = nc.dram_tensor("pos_dram", [1, T], f16)
    cc_in = nc.dram_tensor("cc_in", [1, TS], f32)
    cc_out = nc.dram_tensor("cc_out", [1, T], f32, addr_space="Shared")

    # inline consts
    cc_i = np.arange(8)[None, :, None]
    e_i = np.arange(2)[None, None, :]
    p_i = np.arange(P)[:, None, None]
    wrow_np = (cc_i * 256 + 2 * p_i + e_i + 1.0).reshape(P, 16).astype(np.float32)
    wrow_d = nc.inline_tensor(wrow_np, "wrow")   # attn row weights, i = cc*256+2p+e
    recipd_d = nc.inline_tensor(
        (1.0 / np.arange(T, 0, -1, dtype=np.float32))[None, :], "recipd")
    tio_d = nc.inline_tensor(np.arange(T, dtype=np.float16)[None, :], "tiorow")
    qiota_np = (np.arange(P)[:, None] * 4 + np.arange(4)[None, :]).astype(np.float32)
    qiota_d = nc.inline_tensor(qiota_np, "qiota")  # q = p*4 + j
    mstep_d = nc.inline_tensor(
        ((np.arange(P) + 1.0) / 129.0).astype(np.float32)[:, None], "mstep")

    with tile.TileContext(nc) as tc:
        with tc.tile_pool(name="const", bufs=1) as cpool, \
             tc.tile_pool(name="attn", bufs=3) as apool, \
             tc.tile_pool(name="big", bufs=1) as bpool, \
             tc.tile_pool(name="xp", bufs=1) as xpool, \
             tc.tile_pool(name="io", bufs=4) as iopool, \
             tc.tile_pool(name="sm", bufs=2) as sm, \
             tc.tile_pool(name="st", bufs=1) as st:

            # ---------------- constants ----------------
            SU128 = cpool.tile([P, P], f32, tag="su128")
            make_upper_triangular(nc, SU128[:], val=1.0, diag=False)
            ID128 = cpool.tile([P, P], f32, tag="id128")
            make_identity(nc, ID128[:])
            I2 = cpool.tile([2, 2], f32, tag="i2")
            make_identity(nc, I2[:])
            ones128 = cpool.tile([P, P], f32, tag="ones128")
            nc.vector.memset(ones128[:], 1.0)
            onesrow = cpool.tile([1, P], f32, tag="onesrow")
            nc.vector.memset(onesrow[:], 1.0)
            wrow = cpool.tile([P, F], f32, tag="wrow")
            nc.scalar.dma_start(out=wrow[:], in_=wrow_d[:])
            recipd = cpool.tile([1, T], f32, tag="recipd")
            nc.scalar.dma_start(out=recipd[:], in_=recipd_d[:])
            qiota = cpool.tile([P, 4], f32, tag="qiota")
            nc.scalar.dma_start(out=qiota[:], in_=qiota_d[:])
            mstep = cpool.tile([P, 1], f32, tag="mstep")
            nc.scalar.dma_start(out=mstep[:], in_=mstep_d[:])
            tio_rep = cpool.tile([P, T], f16, tag="tiorep")
            w1r = cpool.tile([P, HID * NF], f32, tag="w1r")
            nc.scalar.dma_start(out=w1r[:], in_=w1r_in[:])
            b1r = cpool.tile([P, HID], f32, tag="b1r")
            nc.scalar.dma_start(out=b1r[:], in_=b1r_in[:])
            w2r = cpool.tile([P, HID], f32, tag="w2r")
            nc.scalar.dma_start(out=w2r[:], in_=w2r_in[:])
            b2r = cpool.tile([P, 1], f32, tag="b2r")
            nc.scalar.dma_start(out=b2r[:], in_=b2r_in[:])

            feats = st.tile([P, F * NF], f32, tag="feats")  # [128,16,5]
            feats3 = feats[:].rearrange("p (c f) -> p c f", f=NF)
            trash = st.tile([P, M], f32, tag="bigtrash")  # shared scratch
            trash2 = trash[:, :F * D]
            trash3 = trash2.rearrange("p (r d) -> p r d", d=D)

            # ---------------- x norm (sharded) + allgather ----------------
            hp_ctx = tc.high_priority()
            hp_ctx.__enter__()
            xn = st.tile([P, 2], f32, tag="xn")
            for c in range(2):
                xc = xpool.tile([P, M], f32, tag="xc", bufs=2)
                nc.sync.dma_start(out=xc[:], in_=x_in[c * P:(c + 1) * P, :])
                nc.vector.scalar_tensor_tensor(
                    out=trash[:], in0=xc[:], scalar=0.0, in1=xc[:],
                    op0=mybir.AluOpType.add, op1=mybir.AluOpType.mult,
                    accum_out=xn[:, c:c + 1])
            xnr = st.tile([P, 2], f32, tag="xnr")
            nc.scalar.sqrt(xnr[:], xn[:])
            for c in range(2):
                nc.gpsimd.dma_start(out=cc_in[0, c * P:(c + 1) * P],
                                    in_=xnr[:, c:c + 1])
            nc.gpsimd.collective_compute(
                "AllGather", mybir.AluOpType.bypass,
                replica_groups=[list(range(W))],
                ins=[cc_in[:]], outs=[cc_out[:]])
            cc_rm = cc_out[:].rearrange("one (p r) -> one p r", p=P, r=F)
            nc.scalar.dma_start(out=feats3[:, :, 4], in_=cc_rm[0])
            hp_ctx.__exit__(None, None, None)

            # ---------------- attn weighted colsum ----------------
            with tc.tile_pool(name="psA", bufs=1, space="PSUM") as psA:
                as_ps = [psA.tile([1, 512], f32, tag=f"asps{n}",
                                  name=f"asps{n}") for n in range(4)]
                for cc in range(8):
                    at = apool.tile([P, 2 * T], f32, tag="at", bufs=2)
                    at3 = at[:].rearrange("p (e t) -> p e t", t=T)
                    nc.sync.dma_start(
                        out=at3,
                        in_=attn_in[cc * 256:(cc + 1) * 256, :].rearrange(
                            "(p e) t -> p e t", p=P, e=2))
                    for e in range(2):
                        for n in range(4):
                            nc.tensor.matmul(
                                out=as_ps[n][:],
                                lhsT=wrow[:, cc * 2 + e:cc * 2 + e + 1],
                                rhs=at3[:, e, n * 512:(n + 1) * 512],
                                start=(cc == 0 and e == 0),
                                stop=(cc == 7 and e == 1))
                as_row = st.tile([1, T], f32, tag="asrow")
                for n in range(4):
                    nc.vector.tensor_tensor(
                        out=as_row[:, n * 512:(n + 1) * 512], in0=as_ps[n][:],
                        in1=recipd[:, n * 512:(n + 1) * 512],
                        op=mybir.AluOpType.mult)
            nc.scalar.dma_start(out=as_dram[:], in_=as_row[:])
            as_rm = as_dram[:].rearrange("one (p r) -> one p r", p=P, r=F)
            nc.scalar.dma_start(out=feats3[:, :, 0], in_=as_rm[0])

            # ---------------- k/v/q norms (1MB contiguous loads) ----------
            for name_, src, fi in (("kn", k_in, 1), ("vn", v_in, 2)):
                kb = bpool.tile([P, F * D], f32, tag="kb", bufs=2)
                kb3 = kb[:].rearrange("p (r d) -> p r d", d=D)
                nc.sync.dma_start(
                    out=kb3, in_=src[:].rearrange("(p r) d -> p r d", p=P, r=F))
                nc.vector.tensor_tensor(out=trash2, in0=kb[:], in1=kb[:],
                                        op=mybir.AluOpType.mult)
                n2 = st.tile([P, F], f32, tag=f"{name_}2", name=f"{name_}2")
                nc.vector.tensor_reduce(out=n2[:], in_=trash3,
                                        axis=mybir.AxisListType.X,
                                        op=mybir.AluOpType.add)
                nc.scalar.sqrt(feats3[:, :, fi], n2[:])
            qs = []
            for g in range(G):
                qt = bpool.tile([P, F * D], f32, tag=f"qt{g}", name=f"qt{g}")
                nc.sync.dma_start(
                    out=qt[:].rearrange("p (r d) -> p r d", d=D),
                    in_=q_in[g].rearrange("(p r) d -> p r d", p=P, r=F))
                qs.append(qt)
            nc.vector.tensor_tensor(out=qs[0][:], in0=qs[0][:], in1=qs[1][:],
                                    op=mybir.AluOpType.add)
            nc.vector.tensor_tensor(out=qs[2][:], in0=qs[2][:], in1=qs[3][:],
                                    op=mybir.AluOpType.add)
            nc.vector.tensor_tensor(out=qs[0][:], in0=qs[0][:], in1=qs[2][:],
                                    op=mybir.AluOpType.add)
            nc.vector.tensor_tensor(out=trash2, in0=qs[0][:], in1=qs[0][:],
                                    op=mybir.AluOpType.mult)
            qn2 = st.tile([P, F], f32, tag="qn2")
            nc.vector.tensor_reduce(out=qn2[:], in_=trash3,
                                    axis=mybir.AxisListType.X,
                                    op=mybir.AluOpType.add)
            nc.scalar.activation(feats3[:, :, 3], qn2[:],
                                 mybir.ActivationFunctionType.Sqrt,
                                 scale=1.0 / (G * G))

            # ---------------- MLP ----------------
            h3d = st.tile([P, F * HID], f32, tag="h3d")
            h3 = h3d[:].rearrange("p (c j) -> p c j", j=HID)
            mul4 = sm.tile([P, F * 4], f32, tag="mul4")
            mul4_3 = mul4[:].rearrange("p (c f) -> p c f", f=4)
            for j in range(HID):
                w1j = _bcast_mid(w1r[:, j * NF + 1:(j + 1) * NF], F)
                nc.vector.tensor_tensor(out=mul4_3, in0=feats3[:, :, 1:5],
                                        in1=w1j, op=mybir.AluOpType.mult)
                nc.vector.tensor_reduce(
                    out=h3[:, :, j], in_=mul4_3, axis=mybir.AxisListType.X,
                    op=mybir.AluOpType.add)
            b1b = _bcast_mid(b1r[:], F)
            nc.vector.tensor_tensor(out=h3, in0=h3, in1=b1b,
                                    op=mybir.AluOpType.add)
            for j in range(HID):
                nc.vector.scalar_tensor_tensor(
                    out=h3[:, :, j], in0=feats3[:, :, 0],
                    scalar=w1r[:, j * NF:j * NF + 1], in1=h3[:, :, j],
                    op0=mybir.AluOpType.mult, op1=mybir.AluOpType.add)
            nc.scalar.activation(h3d[:], h3d[:],
                                 mybir.ActivationFunctionType.Relu)
            mul10 = sm.tile([P, F * HID], f32, tag="mul10")
            mul10_3 = mul10[:].rearrange("p (c j) -> p c j", j=HID)
            w2b = _bcast_mid(w2r[:], F)
            nc.vector.tensor_tensor(out=mul10_3, in0=h3, in1=w2b,
                                    op=mybir.AluOpType.mult)
            pri = st.tile([P, F], f32, tag="pri")
            nc.vector.tensor_reduce(out=pri[:], in_=mul10_3,
                                    axis=mybir.AxisListType.X,
                                    op=mybir.AluOpType.add)
            nc.vector.tensor_scalar(out=pri[:], in0=pri[:],
                                    scalar1=b2r[:, 0:1], scalar2=None,
                                    op0=mybir.AluOpType.add)
            # replicate priorities to [128, T] (each partition = full array)
            nc.scalar.dma_start(out=pri_dram[0:1, :].rearrange(
                "one (p r) -> one p r", p=P, r=F)[0], in_=pri[:])
            pri_rep = st.tile([P, T], f32, tag="prirep")
            nc.sync.dma_start(out=pri_rep[:],
                                in_=bass.AP(pri_dram, 0, [[0, P], [1, T]]))

            from contextlib import ExitStack
            ps_ctx = ExitStack()
            ps = ps_ctx.enter_context(
                tc.tile_pool(name="psB", bufs=1, space="PSUM"))

            def pair_reduce_bcast(pair_t, tag):
                """[128,2] -> PSUM [128,2]: per-column partition-MAX,
                broadcast to every partition (one PE transpose-matmul +
                one DVE reduce + one PE broadcast-matmul)."""
                tp_ps = ps.tile([2, P], f32, tag="tp2", name=f"tp2{tag}")
                nc.tensor.matmul(out=tp_ps[:], lhsT=pair_t[:], rhs=ID128[:],
                                 start=True, stop=True)
                red2 = sm.tile([2, 1], f32, tag="red2", name=f"red2{tag}")
                nc.vector.tensor_reduce(out=red2[:], in_=tp_ps[:],
                                        axis=mybir.AxisListType.X,
                                        op=mybir.AluOpType.max)
                bc_ps = ps.tile([P, 2], f32, tag="bc2", name=f"bc2{tag}")
                nc.tensor.matmul(out=bc_ps[:],
                                 lhsT=red2[:].to_broadcast([2, P]), rhs=I2[:],
                                 start=True, stop=True)
                return bc_ps

            # ---------------- search init (global min/max) ----------------
            cmax = sm.tile([P, 1], f32, tag="cmax")
            nc.vector.tensor_reduce(out=cmax[:], in_=pri[:],
                                    axis=mybir.AxisListType.X,
                                    op=mybir.AluOpType.max)
            cmin = sm.tile([P, 1], f32, tag="cmin")
            nc.vector.tensor_reduce(out=cmin[:], in_=pri[:],
                                    axis=mybir.AxisListType.X,
                                    op=mybir.AluOpType.min)
            hi = st.tile([P, 1], f32, tag="hi")
            lo = st.tile([P, 1], f32, tag="lo")
            ipair = sm.tile([P, 2], f32, tag="pair", name="ipair")
            nc.vector.tensor_copy(out=ipair[:, 0:1], in_=cmax[:])
            nc.vector.tensor_scalar_mul(ipair[:, 1:2], cmin[:], -1.0)
            ibc = pair_reduce_bcast(ipair, "init")
            nc.vector.tensor_copy(out=hi[:], in_=ibc[:, 0:1])
            nc.vector.tensor_scalar_mul(lo[:], ibc[:, 1:2], -1.0)

            def count_gt(th_t, tag):
                """[P,1] count of pri > th (replicated count, no PE)."""
                cnt = sm.tile([P, 1], f32, tag="cnt", name=f"cnt{tag}")
                nc.vector.tensor_scalar(
                    out=trash[:, :T], in0=pri_rep[:], scalar1=th_t[:, 0:1],
                    scalar2=0.0, op0=mybir.AluOpType.is_gt,
                    op1=mybir.AluOpType.add, accum_out=cnt[:])
                return cnt

            def flags_of(cnt, tag):
                flag = sm.tile([P, 1], i32, tag="bflag", name=f"f{tag}")
                nc.vector.tensor_scalar(
                    out=flag[:], in0=cnt[:], scalar1=float(L), scalar2=None,
                    op0=mybir.AluOpType.is_ge)
                nflag = sm.tile([P, 1], i32, tag="bnflag", name=f"nf{tag}")
                nc.vector.tensor_scalar(
                    out=nflag[:], in0=cnt[:], scalar1=float(L), scalar2=None,
                    op0=mybir.AluOpType.is_lt)
                return flag, nflag

            # ---------------- 128-way multiway search ----------------
            for it in range(N_MULTI):
                d = sm.tile([P, 1], f32, tag="mwd", name=f"d{it}")
                nc.vector.tensor_tensor(out=d[:], in0=hi[:], in1=lo[:],
                                        op=mybir.AluOpType.subtract)
                th = sm.tile([P, 1], f32, tag="mwth", name=f"th{it}")
                nc.vector.scalar_tensor_tensor(
                    out=th[:], in0=d[:], scalar=mstep[:, 0:1], in1=lo[:],
                    op0=mybir.AluOpType.mult, op1=mybir.AluOpType.add)
                cnt = count_gt(th, f"m{it}")
                flag, nflag = flags_of(cnt, f"m{it}")
                negth = sm.tile([P, 1], f32, tag="negth", name=f"nt{it}")
                nc.vector.tensor_scalar_mul(negth[:], th[:], -1.0)
                pair = sm.tile([P, 2], f32, tag="pair", name=f"pair{it}")
                nc.vector.memset(pair[:], -3.0e38)
                nc.vector.copy_predicated(pair[:, 0:1], flag[:], th[:])
                nc.vector.copy_predicated(pair[:, 1:2], nflag[:], negth[:])
                bc = pair_reduce_bcast(pair, f"m{it}")
                nc.vector.tensor_tensor(out=lo[:], in0=lo[:], in1=bc[:, 0:1],
                                        op=mybir.AluOpType.max)
                neghi = sm.tile([P, 1], f32, tag="neghi", name=f"nh{it}")
                nc.vector.tensor_scalar_mul(neghi[:], bc[:, 1:2], -1.0)
                nc.vector.tensor_tensor(out=hi[:], in0=hi[:], in1=neghi[:],
                                        op=mybir.AluOpType.min)

            # ---------------- binary polish to 1 ulp ----------------
            for it in range(N_POLISH):
                mid = sm.tile([P, 1], f32, tag="mid", name=f"mid{it}")
                nc.vector.tensor_scalar(
                    out=mid[:], in0=lo[:], scalar1=hi[:, 0:1], scalar2=0.5,
                    op0=mybir.AluOpType.add, op1=mybir.AluOpType.mult)
                cnt = count_gt(mid, f"p{it}")
                flag, nflag = flags_of(cnt, f"p{it}")
                nc.vector.copy_predicated(lo[:], flag[:], mid[:])
                nc.vector.copy_predicated(hi[:], nflag[:], mid[:])

            # ---------------- selection masks ----------------
            strictM = st.tile([P, F], f32, tag="strictM")
            nc.vector.tensor_scalar(
                out=strictM[:], in0=pri[:], scalar1=hi[:, 0:1], scalar2=None,
                op0=mybir.AluOpType.is_gt)
            scnt = count_gt(hi, "s")
            need = st.tile([P, 1], f32, tag="need")
            nc.vector.tensor_scalar(
                out=need[:], in0=scnt[:], scalar1=-1.0, scalar2=float(L),
                op0=mybir.AluOpType.mult, op1=mybir.AluOpType.add)
            eqgt = sm.tile([P, F], f32, tag="eqgt")
            nc.vector.tensor_scalar(
                out=eqgt[:], in0=pri[:], scalar1=lo[:, 0:1], scalar2=None,
                op0=mybir.AluOpType.is_gt)
            eqM = st.tile([P, F], f32, tag="eqM")
            nc.vector.tensor_tensor(out=eqM[:], in0=eqgt[:], in1=strictM[:],
                                    op=mybir.AluOpType.subtract)

            def cumsum_rm(mask_tile, tag):
                """Inclusive cumsum over t (row-major [128,16]) -> SBUF tile."""
                rsum = sm.tile([P, 1], f32, tag="rsum", name=f"{tag}rsum")
                nc.vector.tensor_reduce(out=rsum[:], in_=mask_tile[:],
                                        axis=mybir.AxisListType.X,
                                        op=mybir.AluOpType.add)
                exo_ps = ps.tile([P, 1], f32, tag="exops", name=f"{tag}exo")
                nc.tensor.matmul(out=exo_ps[:], lhsT=SU128[:], rhs=rsum[:],
                                 start=True, stop=True)
                sa = sm.tile([P, 24], f32, tag="scana", name=f"{tag}sa")
                sb_ = sm.tile([P, 24], f32, tag="scanb", name=f"{tag}sb")
                nc.vector.memset(sa[:], 0.0)
                nc.vector.memset(sb_[:], 0.0)
                nc.vector.tensor_copy(out=sa[:, 8:24], in_=mask_tile[:])
                nc.vector.tensor_tensor(out=sb_[:, 8:24], in0=sa[:, 8:24],
                                        in1=sa[:, 7:23], op=mybir.AluOpType.add)
                nc.vector.tensor_tensor(out=sa[:, 8:24], in0=sb_[:, 8:24],
                                        in1=sb_[:, 6:22], op=mybir.AluOpType.add)
                nc.vector.tensor_tensor(out=sb_[:, 8:24], in0=sa[:, 8:24],
                                        in1=sa[:, 4:20], op=mybir.AluOpType.add)
                exo_sb = sm.tile([P, 1], f32, tag="exosb", name=f"{tag}exosb")
                nc.vector.tensor_copy(out=exo_sb[:], in_=exo_ps[:])
                cum = sm.tile([P, F], f32, tag="cum", name=f"{tag}cum")
                nc.vector.tensor_tensor(out=cum[:], in0=sb_[:, 8:24],
                                        in1=sb_[:, 0:16], op=mybir.AluOpType.add)
                cumt = st.tile([P, F], f32, tag=f"{tag}cumt", name=f"{tag}cumt")
                nc.vector.tensor_scalar(
                    out=cumt[:], in0=cum[:], scalar1=exo_sb[:, 0:1],
                    scalar2=None, op0=mybir.AluOpType.add)
                return cumt

            cum_eq = cumsum_rm(eqM, "a")
            leM = sm.tile([P, F], f32, tag="leM")
            nc.vector.tensor_scalar(
                out=leM[:], in0=cum_eq[:], scalar1=need[:, 0:1],
                scalar2=None, op0=mybir.AluOpType.is_le)
            take = sm.tile([P, F], f32, tag="take")
            nc.vector.tensor_tensor(out=take[:], in0=leM[:], in1=eqM[:],
                                    op=mybir.AluOpType.mult)
            sel = st.tile([P, F], f32, tag="sel")
            nc.vector.tensor_tensor(out=sel[:], in0=strictM[:], in1=take[:],
                                    op=mybir.AluOpType.add)

            possel = cumsum_rm(sel, "b")
            posm1 = sm.tile([P, F], f32, tag="posm1")
            nc.vector.tensor_scalar(
                out=posm1[:], in0=possel[:], scalar1=1.0 + BIG,
                scalar2=None, op0=mybir.AluOpType.subtract)
            selpos = sm.tile([P, F], f32, tag="selpos")
            nc.vector.tensor_tensor(out=selpos[:], in0=posm1[:], in1=sel[:],
                                    op=mybir.AluOpType.mult)
            offf = st.tile([P, F], f32, tag="offf")
            nc.vector.tensor_scalar(
                out=offf[:], in0=selpos[:], scalar1=BIG, scalar2=None,
                op0=mybir.AluOpType.add)
            ps_ctx.close()

            nc.scalar.dma_start(out=tio_rep[:],
                                in_=bass.AP(tio_d, 0, [[0, P], [1, T]]))
            # -------- keep indices via compare-sum against positions ------
            offh = sm.tile([P, F], f16, tag="offh")
            nc.vector.tensor_copy(out=offh[:], in_=offf[:])
            nc.scalar.dma_start(out=pos_dram[0:1, :].rearrange(
                "one (p r) -> one p r", p=P, r=F)[0], in_=offh[:])
            pos_rep = st.tile([P, T], f16, tag="posrep")
            nc.sync.dma_start(out=pos_rep[:],
                              in_=bass.AP(pos_dram, 0, [[0, P], [1, T]]))
            keepf = st.tile([P, 4], f32, tag="keepf")
            for j in range(4):
                nc.vector.scalar_tensor_tensor(
                    out=trash[:, :T], in0=pos_rep[:], scalar=qiota[:, j:j + 1],
                    in1=tio_rep[:], op0=mybir.AluOpType.is_equal,
                    op1=mybir.AluOpType.mult, accum_out=keepf[:, j:j + 1])
            keepi = st.tile([P, 4], i32, tag="keepi")
            nc.vector.tensor_copy(out=keepi[:], in_=keepf[:])
            nc.gpsimd.dma_start(
                out=bass.AP(keep_out, 0, [[4, P], [1, 4]]), in_=keepi[:])

            # ---------------- gather k/v rows ----------------
            for src, dst in ((k_in, ko_out), (v_in, vo_out)):
                gall = iopool.tile([P, 4 * D], f32, tag="gall")
                for j in range(4):
                    nc.gpsimd.indirect_dma_start(
                        out=gall[:, j * D:(j + 1) * D], out_offset=None,
                        in_=src[:],
                        in_offset=bass.IndirectOffsetOnAxis(
                            ap=keepi[:, j:j + 1], axis=0))
                nc.sync.dma_start(
                    out=bass.AP(dst, 0, [[4 * D, P], [D, 4], [1, D]]),
                    in_=gall[:].rearrange("p (j d) -> p j d", d=D))

            if debug:
                nc.sync.dma_start(out=dbg_feats[:], in_=feats[:])
                nc.sync.dma_start(out=dbg_pri[:], in_=pri[:])
                nc.sync.dma_start(out=dbg_bounds[:, 0:1], in_=lo[:])
                nc.sync.dma_start(out=dbg_bounds[:, 1:2], in_=hi[:])
                nc.sync.dma_start(out=dbg_sel[:], in_=sel[:])
                nc.sync.dma_start(out=dbg_offf[:], in_=offf[:])
                nc.sync.dma_start(out=dbg_keepf[:], in_=keepf[:])


    _fix_multi_waits(nc)
    return nc


def kernel(input_pos, k_val, v_val, query, x, attn, W1, b1, W2, b2, **_):
    global last_results
    k_val = np.asarray(k_val, dtype=np.float32)
    v_val = np.asarray(v_val, dtype=np.float32)
    query = np.asarray(query, dtype=np.float32)
    x = np.asarray(x, dtype=np.float32)
    attn = np.asarray(attn, dtype=np.float32)
    W1 = np.asarray(W1, dtype=np.float32)
    b1 = np.asarray(b1, dtype=np.float32)
    W2 = np.asarray(W2, dtype=np.float32)
    b2 = np.asarray(b2, dtype=np.float32)

    nc = _build()
    in_maps = []
    for h in range(W):
        in_maps.append({
            "attn": np.ascontiguousarray(attn[0, h]),
            "k": np.ascontiguousarray(k_val[0, h]),
            "v": np.ascontiguousarray(v_val[0, h]),
            "q": np.ascontiguousarray(query[0, G * h:G * (h + 1)]),
            "xs": np.ascontiguousarray(x[0, TS * h:TS * (h + 1)]),
            "w1r": np.broadcast_to(
                W1[h].T.reshape(1, HID * NF), (P, HID * NF)).copy(),
            "b1r": np.broadcast_to(b1[h][None, :], (P, HID)).copy(),
            "w2r": np.broadcast_to(W2[h][:, 0][None, :], (P, HID)).copy(),
            "b2r": np.broadcast_to(b2[h][None, :], (P, 1)).copy(),
        })
    res = run_bass_kernel_spmd(nc, in_maps, list(range(W)))
    last_results = res

    keep = np.stack([res.results[h]["keep"][:, 0] for h in range(W)]).astype(np.int32)
    k_out = np.stack([res.results[h]["ko"] for h in range(W)])[None]
    v_out = np.stack([res.results[h]["vo"] for h in range(W)])[None]
    return keep, k_out, v_out
